# revision 2
# baseline (speedup 1.0000x reference)
"""DenseGCNLayer (GCNConv + BatchNorm + ReLU) on 8 TRN2 NeuronCores.

Self-contained kernel: takes the FULL inputs, shards nodes across 8 cores,
runs a raw-bass SPMD program (bf16 compute, f32 accumulation), returns the
full [N, D] float32 output.

Math: with g[r] = (x @ W.T)[r] * dinv[r] and dinv = rsqrt(indeg + 1),
  agg[c]  = dinv[c] * (sum_{r->c} g[r] + g[c])        (bias cancels in BN)
  y       = relu(A * agg + B),  A = gamma*rsqrt(var+eps), B = beta - A*mean
where mean/var are batch stats of agg over all nodes (all-reduced).

Per core: nodes are tiled by 128 targets; incoming edges are grouped per
tile, split by source-id half (int16 index limit), padded to 128-edge
blocks, and streamed via dma_gather (4 SWDGE queues). Each 128-edge block
is reduced into its target tile by a PE matmul with a 0/1 "one-hot"
(edge-slot -> target-slot) matrix built on DVE from host-provided slot ids.
"""
from contextlib import ExitStack

import numpy as np
import ml_dtypes

import concourse.bass as bass
import concourse.bacc as bacc
import concourse.mybir as mybir
from concourse.library_config import mlp

P = 128
GK = 8          # gather blocks per dma_gather instruction
NIDX = GK * P   # 1024 indices per gather instruction
GW = 4          # one-hot blocks built per DVE op
RL = 6          # gather ring depth (chunks) per stream
NSEM = 8        # rotating DMA sems per stream (> RL => unambiguous)
OHR = 16        # one-hot ring depth (groups)
NQ = 4          # SWDGE queues
XR = 4          # xT / out ring depth
SQR = 4         # sq ring
BN_EPS = 1e-5
BF16 = ml_dtypes.bfloat16
NCONST = 7      # small const loads (Wt x2, gb, iota, onesrow, onesf, deg)


# ---------------------------------------------------------------- host prep

def _preprocess(x, edge_index, W, gamma, beta, M=8):
    N, D = x.shape
    S = N // M
    assert S * M == N
    T = (S + P - 1) // P
    TS = T * P
    NH = N // 2
    src = np.asarray(edge_index[0], np.int64)
    tgt = np.asarray(edge_index[1], np.int64)
    deg = (np.bincount(tgt, minlength=N) + 1).astype(np.float32)

    core_of = tgt // S
    loc = tgt - core_of * S
    tl = loc // P
    slot = loc % P
    ishi = (src >= NH).astype(np.int64)
    key = (core_of * T + tl) * 2 + ishi
    order = np.argsort(key, kind="stable")
    cnt = np.bincount(key, minlength=M * T * 2).reshape(M, T, 2)
    starts = np.zeros(M * T * 2 + 1, np.int64)
    np.cumsum(cnt.reshape(-1), out=starts[1:])

    nblk = -(-cnt // P)                       # ceil, [M, T, 2]
    Blo = np.maximum(nblk[:, :, 0].max(axis=0), 1)   # [T]
    Bhi = np.maximum(nblk[:, :, 1].max(axis=0), 1)
    Blo[T - 1] += (-Blo.sum()) % GK
    Bhi[T - 1] += (-Bhi.sum()) % GK
    LB, HB = int(Blo.sum()), int(Bhi.sum())
    LC, HC = LB // GK, HB // GK
    lo_start = np.zeros(T, np.int64); np.cumsum(Blo[:-1], out=lo_start[1:])
    hi_start = np.zeros(T, np.int64); np.cumsum(Bhi[:-1], out=hi_start[1:])

    # consumption order: per tile, lo blocks then hi blocks
    cons = []
    for t in range(T):
        for i in range(int(Blo[t])):
            cons.append((0, int(lo_start[t] + i), t))
        for i in range(int(Bhi[t])):
            cons.append((1, int(hi_start[t] + i), t))
    NBLK = len(cons)

    # pool issue order: chunks sorted by first consumption step
    first_need = {}
    for step, (st, sp, _t) in enumerate(cons):
        ch = (st, sp // GK)
        if ch not in first_need:
            first_need[ch] = step
    issue_order = sorted(first_need, key=first_need.get)
    assert len(issue_order) == LC + HC

    shared = {}
    shared["Wt"] = np.ascontiguousarray(W.T.astype(BF16)).reshape(2, P, D)
    gb = np.zeros((1, 2 * D), np.float32)
    gb[0, :D], gb[0, D:] = gamma, beta
    shared["gb"] = gb
    shared["iota"] = np.ascontiguousarray(
        np.broadcast_to(np.tile(np.arange(P), GW).astype(BF16), (P, GW * P))
    )
    shared["onesrow"] = np.ones((1, P), np.float32)

    in_maps = []
    for m in range(M):
        lo_src = np.zeros(LB * P, np.int16)
        lo_slot = np.full(LB * P, -1.0, np.float32)
        hi_src = np.zeros(HB * P, np.int16)
        hi_slot = np.full(HB * P, -1.0, np.float32)
        for t in range(T):
            for h, bsrc, bslot, bstart in (
                (0, lo_src, lo_slot, lo_start[t]),
                (1, hi_src, hi_slot, hi_start[t]),
            ):
                k = (m * T + t) * 2 + h
                e = order[starts[k] : starts[k + 1]]
                n = len(e)
                off = int(bstart) * P
                bsrc[off : off + n] = (src[e] - (NH if h else 0)).astype(np.int16)
                bslot[off : off + n] = slot[e].astype(np.float32)

        def wrap_idx(flat, C):
            # per chunk: idx j -> partition j%16, col j//16; replicate x8
            a = flat.reshape(C, NIDX // 16, 16).transpose(0, 2, 1)  # [C,16,cols]
            out = np.concatenate([a] * 8, axis=1)                   # [C,128,cols]
            return np.ascontiguousarray(
                out.transpose(1, 0, 2).reshape(P, C * (NIDX // 16))
            )

        seg = np.full((P, NBLK), -1.0, np.float32)
        for g_blk, (st, sp, _t) in enumerate(cons):
            arr = lo_slot if st == 0 else hi_slot
            seg[:, g_blk] = arr[sp * P : (sp + 1) * P]

        deg_t = np.ones((P, T), np.float32)
        dm = deg[m * S : (m + 1) * S]
        for t in range(T):
            rows = min(P, S - t * P)
            deg_t[:rows, t] = dm[t * P : t * P + rows]

        xT = np.zeros((D, TS), np.float32)
        xT[:, :S] = x[m * S : (m + 1) * S].T
        xT = np.ascontiguousarray(xT).astype(BF16).reshape(2, P, TS)

        onesf = np.zeros((P, 2), np.float32)
        onesf[:, 0] = 1.0
        onesf[: S - (T - 1) * P, 1] = 1.0

        in_maps.append(
            dict(
                xT=xT, idx_lo=wrap_idx(lo_src, LC), idx_hi=wrap_idx(hi_src, HC),
                seg=seg.astype(BF16), deg_t=deg_t, onesf=onesf, **shared,
            )
        )

    meta = dict(
        N=N, D=D, M=M, S=S, T=T, TS=TS, NH=NH,
        Blo=Blo, Bhi=Bhi, LB=LB, HB=HB, LC=LC, HC=HC,
        lo_start=lo_start, hi_start=hi_start, cons=cons,
        issue_order=issue_order,
    )
    return in_maps, meta


# ------------------------------------------------------------- bass program

def _build_program(meta, REP=1):
    N, D, S, T, TS = meta["N"], meta["D"], meta["S"], meta["T"], meta["TS"]
    NH, LC, HC, M = meta["NH"], meta["LC"], meta["HC"], meta["M"]
    cons, issue_order = meta["cons"], meta["issue_order"]
    Blo, Bhi = meta["Blo"], meta["Bhi"]
    lo_start, hi_start = meta["lo_start"], meta["hi_start"]
    LB, HB = meta["LB"], meta["HB"]
    NBLK = len(cons)
    NGRP = -(-NBLK // GW)
    bf = mybir.dt.bfloat16
    f32 = mybir.dt.float32
    ICOL = NIDX // 16

    nc = bacc.Bacc(num_devices=M, num_swdge_queues=NQ, detect_race_conditions=False)

    xT_d = nc.declare_dram_parameter("xT", [2, P, TS], bf, isOutput=False)
    idx_lo_d = nc.declare_dram_parameter("idx_lo", [P, LC * ICOL], mybir.dt.int16, isOutput=False)
    idx_hi_d = nc.declare_dram_parameter("idx_hi", [P, HC * ICOL], mybir.dt.int16, isOutput=False)
    seg_d = nc.declare_dram_parameter("seg", [P, NBLK], bf, isOutput=False)
    deg_d = nc.declare_dram_parameter("deg_t", [P, T], f32, isOutput=False)
    onesf_d = nc.declare_dram_parameter("onesf", [P, 2], f32, isOutput=False)
    Wt_d = nc.declare_dram_parameter("Wt", [2, P, D], bf, isOutput=False)
    gb_d = nc.declare_dram_parameter("gb", [1, 2 * D], f32, isOutput=False)
    iota_d = nc.declare_dram_parameter("iota", [P, GW * P], bf, isOutput=False)
    onesrow_d = nc.declare_dram_parameter("onesrow", [1, P], f32, isOutput=False)
    y_d = nc.declare_dram_parameter("y", [S, D], f32, isOutput=True)

    ag_in = nc.dram_tensor("ag_in", [S, D], bf)
    g_full = nc.dram_tensor("g_full", [N, D], bf, addr_space="Shared")
    st_in = nc.dram_tensor("st_in", [2, D], f32)
    st_out = nc.dram_tensor("st_out", [2, D], f32, addr_space="Shared")

    with ExitStack() as ctx:
        sb = lambda name, shape, dt: ctx.enter_context(nc.sbuf_tensor(name, shape, dt))
        g_sb = sb("g_sb", [P, T * D], bf)
        agg_sb = sb("agg_sb", [P, T * D], f32)
        idx_lo_sb = sb("idx_lo_sb", [P, LC * ICOL], mybir.dt.int16)
        idx_hi_sb = sb("idx_hi_sb", [P, HC * ICOL], mybir.dt.int16)
        seg_sb = sb("seg_sb", [P, NBLK], bf)
        dinv_sb = sb("dinv_sb", [P, T], f32)
        rdeg_sb = sb("rdeg_sb", [P, T], f32)
        deg_sb = sb("deg_sb", [P, T], f32)
        onesf_sb = sb("onesf_sb", [P, 2], f32)
        Wt_sb = sb("Wt_sb", [P, 2 * D], bf)
        gb_sb = sb("gb_sb", [1, 2 * D], f32)
        iota_sb = sb("iota_sb", [P, GW * P], bf)
        onesrow_sb = sb("onesrow_sb", [1, P], f32)
        xT_sb = [sb(f"xT{r}", [P, D], bf) for r in range(XR)]
        ring_lo = [sb(f"rlo{r}", [P, GK * D], bf) for r in range(RL)]
        ring_hi = [sb(f"rhi{r}", [P, GK * D], bf) for r in range(RL)]
        oh_sb = [sb(f"oh{r}", [P, GW * P], bf) for r in range(OHR)]
        sq_sb = [sb(f"sq{r}", [P, D], f32) for r in range(SQR)]
        tmp_e = [sb(f"tmpe{r}", [P, D], f32) for r in range(2)]
        out_sb = [sb(f"out{r}", [P, D], f32) for r in range(XR)]
        t1_sb = sb("t1_sb", [1, D], f32)
        t2_sb = sb("t2_sb", [1, D], f32)
        s1t_sb = sb("s1t_sb", [1, D], f32)
        s2t_sb = sb("s2t_sb", [1, D], f32)
        m1_sb = sb("m1_sb", [1, D], f32)
        var_sb = sb("var_sb", [1, D], f32)
        rstd_sb = sb("rstd_sb", [1, D], f32)
        ab_sb = sb("ab_sb", [1, 2 * D], f32)
        ab128_sb = sb("ab128_sb", [P, 2 * D], f32)

        ph = [ctx.enter_context(nc.psum_tensor(f"ph{r}", [P, D], f32)) for r in range(2)]
        pagg = [ctx.enter_context(nc.psum_tensor(f"pagg{r}", [P, D], f32)) for r in range(2)]
        ps1 = ctx.enter_context(nc.psum_tensor("ps1", [1, D], f32))
        ps2 = ctx.enter_context(nc.psum_tensor("ps2", [1, D], f32))
        pab = ctx.enter_context(nc.psum_tensor("pab", [P, 2 * D], f32))

        sem = lambda name: ctx.enter_context(nc.semaphore(name))
        s_ld = sem("s_ld")        # 6 small const loads
        s_ld2 = sem("s_ld2")      # seg, idx_lo, idx_hi
        s_x = [sem(f"s_x{r}") for r in range(4)]
        s_h = sem("s_h")
        s_g = sem("s_g")
        s_gst = sem("s_gst")
        cc = sem("cc")
        s_glo = [sem(f"s_glo{r}") for r in range(NSEM)]
        s_ghi = [sem(f"s_ghi{r}") for r in range(NSEM)]
        s_clo = sem("s_clo")
        s_chi = sem("s_chi")
        s_oh = sem("s_oh")
        s_ohc = sem("s_ohc")
        s_pa = sem("s_pa")
        s_pac = sem("s_pac")
        s_agg = sem("s_agg")
        s_sq = sem("s_sq")
        s_sqc = sem("s_sqc")
        s_st = sem("s_st")
        s_stsb = sem("s_stsb")
        s_stst = sem("s_stst")
        s_ldst = sem("s_ldst")
        s_var = sem("s_var")
        s_rstd = sem("s_rstd")
        s_ab = sem("s_ab")
        s_abp = sem("s_abp")
        s_y = sem("s_y")
        s_yr = sem("s_yr")
        s_yst = [sem(f"s_yst{r}") for r in range(XR)]
        s_dinv = sem("s_dinv")
        s_di1 = sem("s_di1")
        b1 = ctx.enter_context(nc.semaphore("b1"))
        b2 = ctx.enter_context(nc.semaphore("b2"))
        work_sems = (
            [s_ld, s_ld2, s_h, s_g, s_gst, cc, s_clo, s_chi, s_oh, s_ohc,
             s_pa, s_pac, s_agg, s_sq, s_sqc, s_st, s_stsb, s_stst, s_ldst,
             s_var, s_rstd, s_ab, s_abp, s_y, s_yr, s_dinv, s_di1]
            + s_x + s_glo + s_ghi + s_yst
        )

        def _barrier(eng, it, clear=False):
            if REP == 1:
                return
            eng.sem_inc(b1, 1)
            eng.wait_ge(b1, 5 * (it + 1))
            if clear:
                for ws in work_sems:
                    eng.sem_clear(ws)
                eng.sem_inc(b2, 1)
            eng.wait_ge(b2, it + 1)

        block = ctx.enter_context(nc.Block())

        def block_rhs(st, sp):
            ring = ring_lo if st == 0 else ring_hi
            ch = sp // GK
            return ring[ch % RL][:, (sp % GK) * D : (sp % GK + 1) * D], ch

        # ---------------- SP: all HWDGE loads/stores
        @block.sync
        def _(sync):
            for _it in range(REP):
                for k in range(2):
                    sync.dma_start(
                        out=Wt_sb[:, k * D : (k + 1) * D], in_=Wt_d[k]
                    ).then_inc(s_ld, 16)
                for dram, sbuf in (
                    (gb_d, gb_sb),
                    (iota_d, iota_sb),
                    (onesrow_d, onesrow_sb),
                    (onesf_d, onesf_sb),
                    (deg_d, deg_sb),
                ):
                    sync.dma_start(out=sbuf[:], in_=dram[:]).then_inc(s_ld, 16)
                for t in range(T):
                    if t >= XR:
                        sync.wait_ge(s_h, t - XR + 1)
                    for k in range(2):
                        sync.dma_start(
                            out=xT_sb[t % XR][:, k * P : (k + 1) * P],
                            in_=xT_d[k, :, t * P : (t + 1) * P],
                        ).then_inc(s_x[t % 4], 16)
                for dram, sbuf in (
                    (seg_d, seg_sb),
                    (idx_lo_d, idx_lo_sb),
                    (idx_hi_d, idx_hi_sb),
                ):
                    sync.dma_start(out=sbuf[:], in_=dram[:]).then_inc(s_ld2, 16)
                for t in range(T):
                    rows = min(P, S - t * P)
                    sync.wait_ge(s_g, t + 1)
                    sync.dma_start(
                        out=ag_in[t * P : t * P + rows, :],
                        in_=g_sb[:rows, t * D : (t + 1) * D],
                    ).then_inc(s_gst, 16)
                sync.wait_ge(s_stsb, 2)
                sync.dma_start(out=st_in[0:1, :], in_=t1_sb[:]).then_inc(s_stst, 16)
                sync.dma_start(out=st_in[1:2, :], in_=t2_sb[:]).then_inc(s_stst, 16)
                sync.wait_ge(cc, 2)
                sync.dma_start(out=s1t_sb[:], in_=st_out[0:1, :]).then_inc(s_ldst, 16)
                sync.dma_start(out=s2t_sb[:], in_=st_out[1:2, :]).then_inc(s_ldst, 16)
                for t in range(T):
                    rows = min(P, S - t * P)
                    sync.wait_ge(s_yr, t + 1)
                    sync.dma_start(
                        out=y_d[t * P : t * P + rows, :], in_=out_sb[t % XR][:rows, :]
                    ).then_inc(s_yst[t % XR], 16)
                for r in range(XR):
                    cntr = len([t for t in range(T) if t % XR == r])
                    sync.wait_ge(s_yst[r], 16 * cntr)
                _barrier(sync, _it, clear=False)

        # ---------------- Pool: collectives + gathers
        @block.gpsimd
        def _(gpsimd):
            for _it in range(REP):
                gpsimd.load_library(mlp)
                nreg = gpsimd.snap(NIDX)
                gpsimd.wait_ge(s_gst, 16 * T)
                gpsimd.collective_compute(
                    "AllGather",
                    mybir.AluOpType.bypass,
                    replica_groups=[list(range(M))],
                    ins=[ag_in[:]],
                    outs=[g_full[:]],
                ).then_inc(cc, 1)
                gpsimd.wait_ge(cc, 1)
                gpsimd.wait_ge(s_ld2, 48)
                nis = 0
                cnt_issue = [0, 0]
                for st, ch in issue_order:
                    if cnt_issue[st] >= RL:
                        gpsimd.wait_ge(s_clo if st == 0 else s_chi, cnt_issue[st] - RL + 1)
                    idx_sb = idx_lo_sb if st == 0 else idx_hi_sb
                    tbl = g_full[:NH, :] if st == 0 else g_full[NH:, :]
                    ring = ring_lo if st == 0 else ring_hi
                    gpsimd.dma_gather(
                        ring[ch % RL][:].rearrange("p (k d) -> p k d", d=D),
                        tbl,
                        idx_sb[:, ch * ICOL : (ch + 1) * ICOL],
                        NIDX,
                        nreg,
                        D,
                        queue_num=(ch % NSEM) % NQ,
                    ).then_inc((s_glo if st == 0 else s_ghi)[ch % NSEM], 16)
                    nis += 1
                    cnt_issue[st] += 1
                gpsimd.wait_ge(s_stst, 32)
                gpsimd.collective_compute(
                    "AllReduce",
                    mybir.AluOpType.add,
                    replica_groups=[list(range(M))],
                    ins=[st_in[:]],
                    outs=[st_out[:]],
                ).then_inc(cc, 1)
                _barrier(gpsimd, _it, clear=True)

        # ---------------- PE
        @block.tensor
        def _(tensor):
            for _it in range(REP):
                tensor.wait_ge(s_ld, 16 * NCONST)
                for t in range(T):
                    tensor.wait_ge(s_x[t % 4], 32 * (t // 4 + 1))
                    if t >= 2:
                        tensor.wait_ge(s_g, t - 1)
                    tensor.matmul(
                        ph[t % 2][:], xT_sb[t % XR][:, 0:P], Wt_sb[:, 0:D],
                        start=True, stop=False,
                    )
                    tensor.matmul(
                        ph[t % 2][:], xT_sb[t % XR][:, P : 2 * P], Wt_sb[:, D : 2 * D],
                        start=False, stop=True,
                    ).then_inc(s_h, 1)

                waited_ch = [-1, -1]
                g_blk = 0
                pending = []

                def attach(mm, *incs):
                    # matmul sync-update slots are limited to 1; overflow rides
                    # the next matmul (consumers only ever see a later inc).
                    todo = pending + list(incs)
                    pending.clear()
                    for semh, v in todo[:1]:
                        mm.then_inc(semh, v)
                    pending.extend(todo[1:])

                def stats_for(tt):
                    tensor.wait_ge(s_agg, tt + 1)
                    c = 1 if tt == T - 1 else 0
                    mm = tensor.matmul(
                        ps1[:], onesf_sb[:, c : c + 1], agg_sb[:, tt * D : (tt + 1) * D],
                        start=(tt == 0), stop=(tt == T - 1),
                    )
                    attach(mm, *([(s_st, 1)] if tt == T - 1 else []))
                    tensor.wait_ge(s_sq, tt + 1)
                    mm = tensor.matmul(
                        ps2[:], onesf_sb[:, c : c + 1], sq_sb[tt % SQR][:],
                        start=(tt == 0), stop=(tt == T - 1),
                    )
                    attach(mm, (s_sqc, 1), *([(s_st, 1)] if tt == T - 1 else []))

                for t in range(T):
                    nb = int(Blo[t] + Bhi[t])
                    done = 0
                    for st, base, num in ((0, lo_start[t], Blo[t]), (1, hi_start[t], Bhi[t])):
                        for i in range(int(num)):
                            sp = int(base + i)
                            rhs, ch = block_rhs(st, sp)
                            if ch > waited_ch[st]:
                                tensor.wait_ge(
                                    (s_glo if st == 0 else s_ghi)[ch % NSEM],
                                    16 * (ch // NSEM + 1),
                                )
                                waited_ch[st] = ch
                            grp = g_blk // GW
                            if g_blk % GW == 0:
                                tensor.wait_ge(s_oh, grp + 1)
                            if done == 0 and t >= 2:
                                tensor.wait_ge(s_pac, t - 1)
                            lhsT = oh_sb[grp % OHR][:, (g_blk % GW) * P : (g_blk % GW + 1) * P]
                            mm = tensor.matmul(
                                pagg[t % 2][:], lhsT, rhs,
                                start=(done == 0), stop=(done == nb - 1),
                            )
                            incs = []
                            if done == nb - 1:
                                incs.append((s_pa, 1))
                            if g_blk % GW == GW - 1 or g_blk == NBLK - 1:
                                incs.append((s_ohc, 1))
                            if sp % GK == GK - 1:
                                incs.append((s_clo if st == 0 else s_chi, 1))
                            attach(mm, *incs)
                            done += 1
                            g_blk += 1
                    if t >= 2:
                        stats_for(t - 2)
                stats_for(T - 2)
                stats_for(T - 1)
                for semh, v in pending:
                    tensor.drain().then_inc(semh, v)
                pending.clear()
                tensor.wait_ge(s_ab, 1)
                mm = tensor.matmul(pab[:], onesrow_sb[:], ab_sb[:], start=True, stop=True)
                attach(mm, (s_abp, 1))
                for semh, v in pending:
                    tensor.drain().then_inc(semh, v)
                pending.clear()
                _barrier(tensor, _it, clear=False)

        # ---------------- DVE
        @block.vector
        def _(vector):
            for _it in range(REP):
                vector.wait_ge(s_ld, 16 * NCONST)
                vector.reciprocal(rdeg_sb[:], deg_sb[:]).then_inc(s_di1, 1)
                for t in range(T):
                    vector.wait_ge(s_h, t + 1)
                    if t == 0:
                        vector.wait_ge(s_dinv, 1)
                    vector.tensor_mul(
                        g_sb[:, t * D : (t + 1) * D],
                        ph[t % 2][:],
                        dinv_sb[:, t : t + 1].to_broadcast([P, D]),
                    ).then_inc(s_g, 1)

                vector.wait_ge(s_ld2, 48)
                grp_done = 0
                tile_end_grp = []
                acc = 0
                for t in range(T):
                    acc += int(Blo[t] + Bhi[t])
                    tile_end_grp.append(-(-acc // GW))

                def emit_groups(upto):
                    nonlocal grp_done
                    while grp_done < min(upto, NGRP):
                        g0 = grp_done
                        if g0 >= OHR:
                            vector.wait_ge(s_ohc, g0 - OHR + 1)
                        nblocks = min(GW, NBLK - g0 * GW)
                        vector.tensor_tensor(
                            out=oh_sb[g0 % OHR][:, : nblocks * P],
                            in0=seg_sb[:, g0 * GW : g0 * GW + nblocks].to_broadcast(
                                [P, nblocks, P]
                            ),
                            in1=iota_sb[:, : nblocks * P],
                            op=mybir.AluOpType.is_equal,
                        ).then_inc(s_oh, 1)
                        grp_done += 1

                for t in range(T):
                    emit_groups(tile_end_grp[min(t + 1, T - 1)])
                    vector.wait_ge(s_pa, t + 1)
                    vector.tensor_add(
                        agg_sb[:, t * D : (t + 1) * D],
                        pagg[t % 2][:],
                        g_sb[:, t * D : (t + 1) * D],
                    ).then_inc(s_pac, 1)
                    vector.tensor_mul(
                        agg_sb[:, t * D : (t + 1) * D],
                        agg_sb[:, t * D : (t + 1) * D],
                        dinv_sb[:, t : t + 1].to_broadcast([P, D]),
                    ).then_inc(s_agg, 1)
                    if t >= SQR:
                        vector.wait_ge(s_sqc, t - SQR + 1)
                    vector.tensor_mul(
                        sq_sb[t % SQR][:],
                        agg_sb[:, t * D : (t + 1) * D],
                        agg_sb[:, t * D : (t + 1) * D],
                    ).then_inc(s_sq, 1)
                emit_groups(NGRP)

                vector.wait_ge(s_st, 2)
                vector.tensor_copy(out=t1_sb[:], in_=ps1[:]).then_inc(s_stsb, 1)
                vector.tensor_copy(out=t2_sb[:], in_=ps2[:]).then_inc(s_stsb, 1)
                vector.wait_ge(s_ldst, 32)
                vector.tensor_scalar_mul(m1_sb[:], s1t_sb[:], 1.0 / N)
                vector.tensor_scalar_mul(var_sb[:], s2t_sb[:], 1.0 / N)
                vector.tensor_mul(ab_sb[:, 0:D], m1_sb[:], m1_sb[:])
                vector.tensor_sub(var_sb[:], var_sb[:], ab_sb[:, 0:D])
                vector.tensor_scalar_add(var_sb[:], var_sb[:], BN_EPS)
                vector.reciprocal(var_sb[:], var_sb[:]).then_inc(s_var, 1)
                vector.wait_ge(s_rstd, 1)
                vector.tensor_mul(ab_sb[:, 0:D], gb_sb[:, 0:D], rstd_sb[:])
                vector.tensor_mul(m1_sb[:], ab_sb[:, 0:D], m1_sb[:])
                vector.tensor_sub(ab_sb[:, D : 2 * D], gb_sb[:, D : 2 * D], m1_sb[:])
                vector.sem_inc(s_ab, 1)
                vector.wait_ge(s_abp, 1)
                vector.tensor_copy(out=ab128_sb[:], in_=pab[:])
                for t in range(T):
                    if t >= 2:
                        vector.wait_ge(s_yr, t - 1)
                    vector.tensor_mul(
                        tmp_e[t % 2][:],
                        agg_sb[:, t * D : (t + 1) * D],
                        ab128_sb[:, 0:D],
                    )
                    vector.tensor_add(
                        tmp_e[t % 2][:], tmp_e[t % 2][:], ab128_sb[:, D : 2 * D]
                    ).then_inc(s_y, 1)
                _barrier(vector, _it, clear=False)

        # ---------------- ACT
        @block.scalar
        def _(scalar):
            for _it in range(REP):
                scalar.wait_ge(s_di1, 1)
                scalar.activation(
                    dinv_sb[:], rdeg_sb[:], mybir.ActivationFunctionType.Sqrt
                ).then_inc(s_dinv, 1)
                scalar.wait_ge(s_var, 1)
                scalar.activation(
                    rstd_sb[:], var_sb[:], mybir.ActivationFunctionType.Sqrt
                ).then_inc(s_rstd, 1)
                for t in range(T):
                    scalar.wait_ge(s_y, t + 1)
                    if t >= XR:
                        scalar.wait_ge(s_yst[t % XR], 16 * (t // XR))
                    scalar.activation(
                        out_sb[t % XR][:], tmp_e[t % 2][:],
                        mybir.ActivationFunctionType.Relu,
                    ).then_inc(s_yr, 1)
                _barrier(scalar, _it, clear=False)

    nc.compile()
    return nc


# ------------------------------------------------------------------ driver

_CACHE = {}


def build_all(x, edge_index, W, bias, gamma, beta, M=8):
    x = np.asarray(x, np.float32)
    W = np.asarray(W, np.float32)
    gamma = np.asarray(gamma, np.float32)
    beta = np.asarray(beta, np.float32)
    in_maps, meta = _preprocess(x, edge_index, W, gamma, beta, M)
    sig = (x.shape, meta["LB"], meta["HB"], tuple(meta["Blo"]), tuple(meta["Bhi"]))
    if _CACHE.get("sig") != sig:
        _CACHE["nc"] = _build_program(meta)
        _CACHE["sig"] = sig
    return _CACHE["nc"], in_maps, meta


def assemble_output(per_core_results, meta):
    return np.concatenate(
        [per_core_results[m]["y"] for m in range(meta["M"])], axis=0
    ).astype(np.float32)


def kernel(x, edge_index, W, bias, gamma, beta):
    nc, in_maps, meta = build_all(x, edge_index, W, bias, gamma, beta)
    from concourse.bass_utils import run_bass_kernel_spmd

    res = run_bass_kernel_spmd(nc, in_maps, list(range(meta["M"])))
    y = np.concatenate([res.results[m]["y"] for m in range(meta["M"])], axis=0)
    return y.astype(np.float32)

